# revision 19
# baseline (speedup 1.0000x reference)
"""Trainium2 Bass kernel for nn_ClassificationAverageModel.

reference:
    pooled = mean(embeddings[x], axis=1)        # (B, D)
    logits = pooled @ W.T + b                   # (B, C)
    out    = softmax(logits, axis=1)

Strategy v4 (memory-regime):
  softmax(mean_w(E[x]) @ W.T + b) == softmax(sum_w((E @ (W.T/L))[x]) + b)
so each core projects its vocab shard down to class space
(P = E_shard @ W.T / L, bf16) and keeps it in SBUF in the dma_gather
"rank-stripe" layout (row r -> partition r%128, 256B segment r//128).

Phase 1 is fed with host-side PRE-TRANSPOSED bf16 E.T (layout-only
prep): matmuls produce P.T [20 x 512] directly (cheap 20-wide weights
stay stationary), then small PE transposes land the [128 x 20] chunks
in the rank-stripe table. This kills v2's on-device f32 PE transposes
(phase 1: 147us -> ~40us).

Phase 2 gathers tokens with SBUF-source *transposed* dma_gather in
doc-major order. v4 uses PER-GROUP SORTED BUDGETS: docs are sorted by
their max-over-cores token count, 32-doc groups get their exact budget
(computed from the actual x, ceil to 4), and the host un-permutes the
output rows at the end. Cuts padded gather idx ~180K -> ~135K per core.

A ReduceScatter(add) over the [8*20, 512] partial-logit planes gives
each core its 512 (rank-ordered) docs; 4 PE transposes + bias +
softmax finish; host applies the inverse doc permutation.
"""

import numpy as np
import ml_dtypes

import concourse.bass as bass
import concourse.mybir as mybir
import concourse.tile as tile
from concourse import bacc, library_config
from concourse.bass_utils import run_bass_kernel_spmd
from concourse.masks import make_identity
from concourse.vector_clock import ScopedClock

F32 = mybir.dt.float32
BF16 = mybir.dt.bfloat16
I16 = mybir.dt.int16

NCORES = 8
# max idxs per single-packet gather call: transpose mode needs
# num_idxs/16 + 2 descriptors per engine ring, capped at 896.
GSUB = 896


class PatchedTileContext(tile.TileContext):
    """Split the kernel-tail drain's sem waits: walrus TRN2 CTRL codegen
    rejects drain instructions carrying more than ~2 sync waits."""

    def _drain_and_barrier(self, tick_clock, wait_clock):
        drain_inst = self.nc.sync.drain()
        wait_clock.add_sem_waits(
            drain_inst.ins, ScopedClock({None: tick_clock.global_clock})
        )
        si = drain_inst.ins.sync_info
        waits = list(si.on_wait) if si is not None else []
        if len(waits) > 1:
            si.on_wait = waits[:1]
            for w in waits[1:]:
                d2 = self.nc.sync.drain()
                si2 = d2.ins.sync_info
                if si2 is None:
                    d2.ins.sync_info = mybir.SyncInfo(on_wait=[w], on_update=[])
                else:
                    si2.on_wait = [w]
        self.nc.all_engine_barrier()
        popped = self.nc._tile_sem_poison_stack.pop()
        assert popped is self._sem_poison
        self.nc.clear_and_free_semaphores(list(self.sems.allocated().values()))
        self.nc.all_engine_barrier()


def _ceil4(n):
    return max(4, -(-int(n) // 4) * 4)


def _split_calls(n):
    """Split a group's idx count into near-equal single-packet calls
    (<=896, each a multiple of 128)."""
    assert n % 128 == 0
    k = -(-n // GSUB)
    out = []
    for i in range(k):
        take = -(-(n // 128) // (k - i)) * 128
        out.append(take)
        n -= take
    assert n == 0
    return out


class Cfg:
    def __init__(self, budgets, vocab=100000, embed=300, ncls=20, batch=4096,
                 doclen=200):
        assert vocab % NCORES == 0 and batch % (128 * NCORES) == 0
        self.vocab, self.embed, self.ncls = vocab, embed, ncls
        self.batch, self.doclen = batch, doclen
        self.vsh = vocab // NCORES                  # shard rows per core
        self.nch = -(-self.vsh // 128)              # 128-row chunks (98)
        self.tsegs = self.nch + 1                   # + spare zero segment
        self.pad_idx = self.nch * 128               # rows in the spare segment
        self.trows = self.tsegs * 128
        self.kchunks = [(0, 128), (128, 128), (256, embed - 256)]
        self.kpad = 128 * len(self.kchunks)
        self.vpad = self.nch * 128                  # 12544, et col count
        self.vblk = 512                             # phase-1 v block
        self.nblk = -(-self.vpad // self.vblk)      # 25 (last block 256)
        self.gdocs = 32                             # docs per budget group
        assert batch % self.gdocs == 0
        self.ngrp = batch // self.gdocs             # 128
        # budgets: per-group slot count per doc (mult of 4), from actual x
        assert len(budgets) == self.ngrp
        self.budgets = tuple(int(b) for b in budgets)
        self.goff = [0]
        for b in self.budgets:
            self.goff.append(self.goff[-1] + self.gdocs * b)
        self.nslots = self.goff[-1]
        assert self.nslots % 16 == 0
        self.docs_out = batch // NCORES
        # output chunks: 128 docs of each core's slice, RS'd + softmaxed as
        # soon as their 32 rank-groups (4 per output slice) are pooled
        self.ochunks = self.docs_out // 128          # 4
        self.gp_chunk = 128 // self.gdocs            # rank-groups per chunk
        # group processing order: chunk-major so RS chunks fire early
        self.gorder = []
        for ch in range(self.ochunks):
            for s in range(NCORES):
                for i in range(self.gp_chunk):
                    self.gorder.append(
                        (s * self.docs_out + ch * 128) // self.gdocs + i)
        assert sorted(self.gorder) == list(range(self.ngrp))
        # flat call list in processing order: (group, off_in_group, size,
        # queue); queues greedily balanced by idx load
        self.calls = []
        qload = [0, 0, 0, 0]
        for g in self.gorder:
            off = 0
            for n in _split_calls(self.gdocs * self.budgets[g]):
                q = min(range(4), key=lambda i: qload[i])
                qload[q] += n
                self.calls.append((g, off, n, q))
                off += n

    def key(self):
        return (self.vocab, self.embed, self.ncls, self.batch, self.doclen,
                self.budgets)


def _build_program(cfg: Cfg):
    c = cfg
    nc = bacc.Bacc("TRN2", target_bir_lowering=False, debug=False,
                   num_devices=NCORES, num_swdge_queues=4)
    et_d = nc.dram_tensor("et_d", [c.kpad, c.vpad], BF16, kind="ExternalInput")
    wt_d = nc.dram_tensor("wt_d", [c.kpad, c.ncls], F32, kind="ExternalInput")
    b_d = nc.dram_tensor("b_d", [128, c.ncls], F32, kind="ExternalInput")
    gidx_d = nc.dram_tensor("gidx_d", [128, c.nslots // 16], I16,
                            kind="ExternalInput")
    out_d = nc.dram_tensor("out", [c.docs_out, c.ncls], F32,
                           kind="ExternalOutput")
    partials_d = [nc.dram_tensor(f"partials{i}", [NCORES * c.ncls, 128], F32)
                  for i in range(c.ochunks)]
    rs_d = [nc.dram_tensor(f"rs{i}", [c.ncls, 128], F32)
            for i in range(c.ochunks)]

    nk = len(c.kchunks)
    with PatchedTileContext(nc) as tc:
        with tc.tile_pool(name="const", bufs=1) as cpool:
            nc.gpsimd.load_library(library_config.mlp)

            ident = cpool.tile([128, 128], F32)
            make_identity(nc, ident[:])

            # W.T [kpad, ncls] -> staged per k-chunk, scaled 1/L -> bf16
            wt_f = cpool.tile([128, nk * c.ncls], F32)
            wtv = wt_f[:].rearrange("p (k n) -> p k n", k=nk)
            for k in range(nk):
                nc.sync.dma_start(
                    out=wtv[:, k, :], in_=wt_d[k * 128:(k + 1) * 128, :])
            wt = cpool.tile([128, nk * c.ncls], BF16)
            nc.scalar.mul(out=wt[:], in_=wt_f[:], mul=1.0 / c.doclen)

            b_t = cpool.tile([128, c.ncls], F32)
            gi_all = cpool.tile([128, c.nslots // 16], I16)

            # ---- the projected table, rank-stripe layout ----
            t_sb = cpool.tile([128, c.trows], BF16)
            nc.vector.memset(t_sb[:], 0.0)

            # ---- phase 1: P.T blocks = (W.T/L).T @ E.T, then transpose
            with (
                tc.tile_pool(name="ep", bufs=3) as epool,
                tc.tile_pool(name="st", bufs=3) as spool,
                tc.tile_pool(name="pa", bufs=3, space="PSUM") as papool,
                tc.tile_pool(name="pb", bufs=4, space="PSUM") as pbpool,
            ):
                for blk in range(c.nblk):
                    v0 = blk * c.vblk
                    w = min(c.vblk, c.vpad - v0)
                    e_t = epool.tile([128, nk * c.vblk], BF16)
                    ev = e_t[:].rearrange("p (k v) -> p k v", k=nk)
                    for k in range(nk):
                        nc.sync.dma_start(
                            out=ev[:, k, :w],
                            in_=et_d[k * 128:(k + 1) * 128, v0:v0 + w])
                    pa = papool.tile([128, c.vblk], F32)
                    for k, (k0, kw) in enumerate(c.kchunks):
                        nc.tensor.matmul(
                            out=pa[:c.ncls, :w],
                            lhsT=wt[:kw, k * c.ncls:(k + 1) * c.ncls],
                            rhs=e_t[:kw, k * c.vblk:k * c.vblk + w],
                            start=(k == 0),
                            stop=(k == nk - 1),
                        )
                    stg = spool.tile([128, c.vblk], F32)
                    nc.scalar.copy(out=stg[:c.ncls, :w], in_=pa[:c.ncls, :w])
                    for s in range(w // 128):
                        pb = pbpool.tile([128, c.ncls], F32)
                        nc.tensor.transpose(
                            out=pb[:, :c.ncls],
                            in_=stg[:c.ncls, s * 128:(s + 1) * 128],
                            identity=ident[:c.ncls, :c.ncls],
                        )
                        seg = blk * (c.vblk // 128) + s
                        nc.vector.tensor_copy(
                            out=t_sb[:, seg * 128:seg * 128 + c.ncls],
                            in_=pb[:, :c.ncls])

            # bias + gather indices ride the sync queue behind the et loads
            # (needed only once gathering starts)
            nc.sync.dma_start(out=b_t[:], in_=b_d[:])
            nc.sync.dma_start(out=gi_all[:], in_=gidx_d[:])

            # ---- phase 2+3: gather + reduce, chunked RS + softmax ----
            pooled = cpool.tile([128, c.batch], F32)
            maxw = c.gdocs * max(c.budgets)
            ngrp_chunk = c.ochunks and len(c.gorder) // c.ochunks
            with (
                tc.tile_pool(name="gw", bufs=4) as gwpool,
                tc.tile_pool(name="sm", bufs=2) as smpool,
                tc.tile_pool(name="sms", bufs=2) as sspool,
                tc.tile_pool(name="tps", bufs=2, space="PSUM") as tpool,
            ):
                def emit_partial(s, ch):
                    """One output-slice's partials for chunk ch -> DRAM, as
                    soon as its 4 rank-groups are pooled (keeps the RS
                    trigger's wait nearly satisfied at the chunk boundary)."""
                    col = s * c.docs_out + ch * 128
                    nc.sync.dma_start(
                        out=partials_d[ch][s * c.ncls:(s + 1) * c.ncls, :],
                        in_=pooled[:c.ncls, col:col + 128])

                def emit_chunk(ch):
                    """RS + bias + softmax for output docs [128ch, 128ch+128)
                    of every core's slice."""
                    nc.gpsimd.collective_compute(
                        "ReduceScatter",
                        mybir.AluOpType.add,
                        replica_groups=[list(range(NCORES))],
                        ins=[partials_d[ch][:]],
                        outs=[rs_d[ch][:]],
                    )
                    rs_sb = smpool.tile([c.ncls, 128], F32)
                    nc.sync.dma_start(out=rs_sb[:], in_=rs_d[ch][:])
                    tp = tpool.tile([128, c.ncls], F32)
                    nc.tensor.transpose(
                        out=tp[:, :c.ncls],
                        in_=rs_sb[:, :],
                        identity=ident[:c.ncls, :c.ncls],
                    )
                    lt = smpool.tile([128, c.ncls], F32)
                    nc.vector.tensor_tensor(out=lt[:], in0=tp[:], in1=b_t[:],
                                            op=mybir.AluOpType.add)
                    nmx = sspool.tile([128, 1], F32)
                    nc.vector.tensor_reduce(out=nmx[:], in_=lt[:],
                                            op=mybir.AluOpType.max,
                                            axis=mybir.AxisListType.X,
                                            negate=True)
                    ex = smpool.tile([128, c.ncls], F32)
                    nc.scalar.activation(out=ex[:], in_=lt[:],
                                         func=mybir.ActivationFunctionType.Exp,
                                         bias=nmx[:], scale=1.0)
                    sm = sspool.tile([128, 1], F32)
                    nc.vector.reduce_sum(out=sm[:], in_=ex[:],
                                         axis=mybir.AxisListType.X)
                    rc = sspool.tile([128, 1], F32)
                    nc.vector.reciprocal(out=rc[:], in_=sm[:])
                    ot = smpool.tile([128, c.ncls], F32)
                    nc.vector.tensor_scalar_mul(out=ot[:], in0=ex[:],
                                                scalar1=rc[:])
                    nc.sync.dma_start(out=out_d[ch * 128:(ch + 1) * 128, :],
                                      in_=ot[:])

                grp_done = -1
                groups_reduced = 0
                chunks_emitted = 0
                g_w = None
                g3 = None
                for (g, off, n, q) in c.calls:
                    if g != grp_done:
                        # new group: fresh tile, sliced to this group's width
                        b = c.budgets[g]
                        g_t = gwpool.tile([128, maxw], BF16)
                        g_w = g_t[:, :c.gdocs * b]
                        g3 = g_w.rearrange("p (s n) -> p s n", s=1)
                        grp_done = g
                    base = c.goff[g]
                    nc.gpsimd.dma_gather(
                        out_ap=g3[:, :, off:off + n],
                        in_ap=t_sb[:],
                        idxs_ap=gi_all[:, (base + off) // 16:
                                       (base + off + n) // 16],
                        num_idxs=n,
                        num_idxs_reg=n,
                        elem_size=128,
                        transpose=True,
                        single_packet=True,
                        queue_num=q,
                        sbuf_tokens_per_rank=128,
                        sbuf_free_dim_per_rank=256,
                        sbuf_free_dim_pad_per_rank=0,
                        sbuf_byte_offset=0,
                    )
                    if off + n == c.gdocs * c.budgets[g]:
                        g3d = g_w.rearrange("p (d t) -> p d t",
                                            t=c.budgets[g])
                        nc.vector.tensor_reduce(
                            out=pooled[:, g * c.gdocs:(g + 1) * c.gdocs],
                            in_=g3d,
                            op=mybir.AluOpType.add,
                            axis=mybir.AxisListType.X)
                        groups_reduced += 1
                        p = groups_reduced - 1
                        ch_p, rem = divmod(p, ngrp_chunk)
                        s_p, i_p = divmod(rem, c.gp_chunk)
                        if i_p == c.gp_chunk - 1:
                            emit_partial(s_p, ch_p)
                        if rem == ngrp_chunk - 1:
                            emit_chunk(chunks_emitted)
                            chunks_emitted += 1
                assert chunks_emitted == c.ochunks
    nc.compile()
    return nc


def _plan(x: np.ndarray):
    """Doc ordering + per-group budgets from the actual token counts.

    Returns (cfg, order) where order[rank] = original doc id; docs are
    processed in rank order so each 32-doc group's budget is the exact
    (ceil-4) max token count over its docs and all cores."""
    x = np.asarray(x).astype(np.int64)
    B, L = x.shape
    vsh = 100000 // NCORES
    flat_v = x.reshape(-1)
    tok_doc = np.repeat(np.arange(B, dtype=np.int64), L)
    core_of = flat_v // vsh
    key = core_of * B + tok_doc
    counts = np.bincount(key, minlength=NCORES * B).reshape(NCORES, B)
    docmax = counts.max(axis=0)
    order = np.argsort(docmax, kind="stable")
    smax = docmax[order]
    budgets = [_ceil4(smax[g * 32 + 31]) for g in range(B // 32)]
    cfg = Cfg(budgets, batch=B, doclen=L)
    return cfg, order


def _prep_index_inputs(cfg: Cfg, x: np.ndarray, order: np.ndarray):
    """Rank-major gather indices (16-wrap int16 per call).
    Returns gidx[8, 128, nslots/16]."""
    c = cfg
    x = np.asarray(x).astype(np.int64)
    flat_v = x.reshape(-1)
    tok_doc = np.repeat(np.arange(c.batch, dtype=np.int64), c.doclen)
    core_of = flat_v // c.vsh
    local = (flat_v - core_of * c.vsh).astype(np.int64)

    rank_of = np.empty(c.batch, np.int64)
    rank_of[order] = np.arange(c.batch, dtype=np.int64)

    key = core_of * c.batch + tok_doc
    counts = np.bincount(key, minlength=NCORES * c.batch)
    ord_t = np.argsort(key, kind="stable")
    key_s = key[ord_t]
    group_start = np.zeros(NCORES * c.batch, np.int64)
    np.cumsum(counts[:-1], out=group_start[1:])
    pos = np.arange(key.size, dtype=np.int64) - group_start[key_s]
    core_s = key_s // c.batch
    rank_s = rank_of[key_s % c.batch]

    g = rank_s // c.gdocs
    budg = np.asarray(c.budgets, np.int64)
    goff = np.asarray(c.goff[:-1], np.int64)
    slot = goff[g] + (rank_s % c.gdocs) * budg[g] + pos
    assert (pos < budg[g]).all()

    # pads round-robin over the 128 zero rows of the spare segment
    gflat = np.broadcast_to(
        c.pad_idx + (np.arange(c.nslots, dtype=np.int64) % 128),
        (NCORES, c.nslots)).copy()
    gflat[core_s, slot] = local[ord_t]

    # 16-wrap per call: within each call, token j -> [j%16, j//16]
    g16 = np.empty((NCORES, 16, c.nslots // 16), np.int16)
    for (grp, off, n, _q) in c.calls:
        a = c.goff[grp] + off
        seg = gflat[:, a:a + n].reshape(NCORES, n // 16, 16)
        g16[:, :, a // 16:(a + n) // 16] = seg.transpose(0, 2, 1)
    gidx = np.tile(g16, (1, 8, 1)).astype(np.int16)   # (8, 128, cols)
    return gidx


_PROGRAM_CACHE: dict = {}


def _get_program(cfg: Cfg):
    k = cfg.key()
    if k not in _PROGRAM_CACHE:
        _PROGRAM_CACHE[k] = _build_program(cfg)
    return _PROGRAM_CACHE[k]


def run(embeddings, W, b, x, trace=False, tmpdir=None):
    embeddings = np.ascontiguousarray(np.asarray(embeddings, dtype=np.float32))
    W = np.ascontiguousarray(np.asarray(W, dtype=np.float32))
    b = np.asarray(b, dtype=np.float32).reshape(1, -1)
    x = np.asarray(x)

    cfg, order = _plan(x)
    gidx = _prep_index_inputs(cfg, x, order)
    nc = _get_program(cfg)

    wt_host = np.zeros((cfg.kpad, cfg.ncls), np.float32)
    wt_host[:cfg.embed] = W.T
    b_tiled = np.tile(b, (128, 1)).astype(np.float32)
    in_maps = []
    for c in range(NCORES):
        e_pad = np.zeros((cfg.vpad, cfg.embed), np.float32)
        e_pad[:cfg.vsh] = embeddings[c * cfg.vsh:(c + 1) * cfg.vsh]
        et = np.zeros((cfg.kpad, cfg.vpad), ml_dtypes.bfloat16)
        et[:cfg.embed] = e_pad.T.astype(ml_dtypes.bfloat16)
        in_maps.append({
            "et_d": et,
            "wt_d": wt_host,
            "b_d": b_tiled,
            "gidx_d": gidx[c],
        })
    res = run_bass_kernel_spmd(nc, in_maps, list(range(NCORES)),
                               trace=trace, tmpdir=tmpdir)
    ranked = np.concatenate([res.results[c]["out"] for c in range(NCORES)],
                            axis=0)
    out = np.empty_like(ranked)
    out[order] = ranked          # rank r held doc order[r]
    return out, res


def kernel(embeddings, W, b, x):
    out, _ = run(embeddings, W, b, x)
    return out


# revision 24
# speedup vs baseline: 1.1218x; 1.1218x over previous
"""Trainium2 Bass kernel for nn_ClassificationAverageModel.

reference:
    pooled = mean(embeddings[x], axis=1)        # (B, D)
    logits = pooled @ W.T + b                   # (B, C)
    out    = softmax(logits, axis=1)

Strategy v4 (memory-regime):
  softmax(mean_w(E[x]) @ W.T + b) == softmax(sum_w((E @ (W.T/L))[x]) + b)
so each core projects its vocab shard down to class space
(P = E_shard @ W.T / L, bf16) and keeps it in SBUF in the dma_gather
"rank-stripe" layout (row r -> partition r%128, 256B segment r//128).

Phase 1 is fed with host-side PRE-TRANSPOSED bf16 E.T (layout-only
prep): matmuls produce P.T [20 x 512] directly (cheap 20-wide weights
stay stationary), then small PE transposes land the [128 x 20] chunks
in the rank-stripe table. This kills v2's on-device f32 PE transposes
(phase 1: 147us -> ~40us).

Phase 2 gathers tokens with SBUF-source *transposed* dma_gather in
doc-major order. v4 uses PER-GROUP SORTED BUDGETS: docs are sorted by
their max-over-cores token count, 32-doc groups get their exact budget
(computed from the actual x, ceil to 4), and the host un-permutes the
output rows at the end. Cuts padded gather idx ~180K -> ~135K per core.

A ReduceScatter(add) over the [8*20, 512] partial-logit planes gives
each core its 512 (rank-ordered) docs; 4 PE transposes + bias +
softmax finish; host applies the inverse doc permutation.
"""

import numpy as np
import ml_dtypes

import concourse.bass as bass
import concourse.mybir as mybir
import concourse.tile as tile
from concourse import bacc, library_config
from concourse.bass_utils import run_bass_kernel_spmd
from concourse.masks import make_identity
from concourse.vector_clock import ScopedClock

F32 = mybir.dt.float32
BF16 = mybir.dt.bfloat16
I16 = mybir.dt.int16

NCORES = 8
# max idxs per single-packet gather call: transpose mode needs
# num_idxs/16 + 2 descriptors per engine ring, capped at 896.
GSUB = 896


class PatchedTileContext(tile.TileContext):
    """Split the kernel-tail drain's sem waits: walrus TRN2 CTRL codegen
    rejects drain instructions carrying more than ~2 sync waits."""

    def _drain_and_barrier(self, tick_clock, wait_clock):
        drain_inst = self.nc.sync.drain()
        wait_clock.add_sem_waits(
            drain_inst.ins, ScopedClock({None: tick_clock.global_clock})
        )
        si = drain_inst.ins.sync_info
        waits = list(si.on_wait) if si is not None else []
        if len(waits) > 1:
            si.on_wait = waits[:1]
            for w in waits[1:]:
                d2 = self.nc.sync.drain()
                si2 = d2.ins.sync_info
                if si2 is None:
                    d2.ins.sync_info = mybir.SyncInfo(on_wait=[w], on_update=[])
                else:
                    si2.on_wait = [w]
        self.nc.all_engine_barrier()
        popped = self.nc._tile_sem_poison_stack.pop()
        assert popped is self._sem_poison
        self.nc.clear_and_free_semaphores(list(self.sems.allocated().values()))
        self.nc.all_engine_barrier()


def _ceil4(n):
    return max(4, -(-int(n) // 4) * 4)


def _split_calls(n):
    """Split a group's idx count into near-equal single-packet calls
    (<=896, each a multiple of 128)."""
    assert n % 128 == 0
    k = -(-n // GSUB)
    out = []
    for i in range(k):
        take = -(-(n // 128) // (k - i)) * 128
        out.append(take)
        n -= take
    assert n == 0
    return out


class Cfg:
    def __init__(self, budgets, vocab=100000, embed=300, ncls=20, batch=4096,
                 doclen=200):
        assert vocab % NCORES == 0 and batch % (128 * NCORES) == 0
        self.vocab, self.embed, self.ncls = vocab, embed, ncls
        self.batch, self.doclen = batch, doclen
        self.vsh = vocab // NCORES                  # shard rows per core
        self.nch = -(-self.vsh // 128)              # 128-row chunks (98)
        self.tsegs = self.nch + 1                   # + spare zero segment
        self.pad_idx = self.nch * 128               # rows in the spare segment
        self.trows = self.tsegs * 128
        self.kchunks = [(0, 128), (128, 128), (256, embed - 256)]
        self.kpad = 128 * len(self.kchunks)
        self.vpad = self.nch * 128                  # 12544, et col count
        self.vblk = 512                             # phase-1 v block
        self.nblk = -(-self.vpad // self.vblk)      # 25 (last block 256)
        self.gdocs = 32                             # docs per budget group
        assert batch % self.gdocs == 0
        self.ngrp = batch // self.gdocs             # 128
        # budgets: per-group slot count per doc (mult of 4), from actual x
        assert len(budgets) == self.ngrp
        self.budgets = tuple(int(b) for b in budgets)
        self.goff = [0]
        for b in self.budgets:
            self.goff.append(self.goff[-1] + self.gdocs * b)
        self.nslots = self.goff[-1]
        assert self.nslots % 16 == 0
        self.docs_out = batch // NCORES
        # output chunks: 128 docs of each core's slice, RS'd + softmaxed as
        # soon as their 32 rank-groups (4 per output slice) are pooled
        self.ochunks = self.docs_out // 128          # 4
        self.gp_chunk = 128 // self.gdocs            # rank-groups per chunk
        # group processing order: chunk-major so RS chunks fire early
        self.gorder = []
        for ch in range(self.ochunks):
            for s in range(NCORES):
                for i in range(self.gp_chunk):
                    self.gorder.append(
                        (s * self.docs_out + ch * 128) // self.gdocs + i)
        assert sorted(self.gorder) == list(range(self.ngrp))
        # flat call list in processing order: (group, off_in_group, size,
        # queue); queues greedily balanced by idx load
        self.calls = []
        qload = [0, 0, 0, 0]
        for g in self.gorder:
            off = 0
            for n in _split_calls(self.gdocs * self.budgets[g]):
                q = min(range(4), key=lambda i: qload[i])
                qload[q] += n
                self.calls.append((g, off, n, q))
                off += n

    def key(self):
        return (self.vocab, self.embed, self.ncls, self.batch, self.doclen,
                self.budgets)


def _build_program(cfg: Cfg):
    c = cfg
    nc = bacc.Bacc("TRN2", target_bir_lowering=False, debug=False,
                   num_devices=NCORES, num_swdge_queues=4)
    et_d = nc.dram_tensor("et_d", [c.kpad, c.vpad], BF16, kind="ExternalInput")
    wt_d = nc.dram_tensor("wt_d", [c.kpad, c.ncls], F32, kind="ExternalInput")
    b_d = nc.dram_tensor("b_d", [128, c.ncls], F32, kind="ExternalInput")
    gidx_d = nc.dram_tensor("gidx_d", [128, c.nslots // 16], I16,
                            kind="ExternalInput")
    out_d = nc.dram_tensor("out", [c.docs_out, c.ncls], F32,
                           kind="ExternalOutput")
    partials_d = [nc.dram_tensor(f"partials{i}", [NCORES * c.ncls, 128], F32)
                  for i in range(c.ochunks)]
    rs_d = [nc.dram_tensor(f"rs{i}", [c.ncls, 128], F32)
            for i in range(c.ochunks)]

    nk = len(c.kchunks)
    with PatchedTileContext(nc) as tc:
        with tc.tile_pool(name="const", bufs=1) as cpool:
            nc.gpsimd.load_library(library_config.mlp)

            ident = cpool.tile([128, 128], F32)
            make_identity(nc, ident[:])

            # W.T [kpad, ncls] -> staged per k-chunk, scaled 1/L -> bf16
            wt_f = cpool.tile([128, nk * c.ncls], F32)
            wtv = wt_f[:].rearrange("p (k n) -> p k n", k=nk)
            for k in range(nk):
                nc.sync.dma_start(
                    out=wtv[:, k, :], in_=wt_d[k * 128:(k + 1) * 128, :])
            wt = cpool.tile([128, nk * c.ncls], BF16)
            nc.scalar.mul(out=wt[:], in_=wt_f[:], mul=1.0 / c.doclen)

            b_t = cpool.tile([128, c.ncls], F32)
            gi_all = cpool.tile([128, c.nslots // 16], I16)

            # ---- the projected table, rank-stripe layout ----
            t_sb = cpool.tile([128, c.trows], BF16)
            nc.vector.memset(t_sb[:], 0.0)

            # ---- phase 1: P.T blocks = (W.T/L).T @ E.T, then transpose
            with (
                tc.tile_pool(name="ep", bufs=3) as epool,
                tc.tile_pool(name="st", bufs=3) as spool,
                tc.tile_pool(name="pa", bufs=3, space="PSUM") as papool,
                tc.tile_pool(name="pb", bufs=4, space="PSUM") as pbpool,
            ):
                for blk in range(c.nblk):
                    v0 = blk * c.vblk
                    w = min(c.vblk, c.vpad - v0)
                    e_t = epool.tile([128, nk * c.vblk], BF16)
                    ev = e_t[:].rearrange("p (k v) -> p k v", k=nk)
                    for k in range(nk):
                        nc.sync.dma_start(
                            out=ev[:, k, :w],
                            in_=et_d[k * 128:(k + 1) * 128, v0:v0 + w])
                    pa = papool.tile([128, c.vblk], F32)
                    for k, (k0, kw) in enumerate(c.kchunks):
                        nc.tensor.matmul(
                            out=pa[:c.ncls, :w],
                            lhsT=wt[:kw, k * c.ncls:(k + 1) * c.ncls],
                            rhs=e_t[:kw, k * c.vblk:k * c.vblk + w],
                            start=(k == 0),
                            stop=(k == nk - 1),
                        )
                    stg = spool.tile([128, c.vblk], F32)
                    nc.scalar.copy(out=stg[:c.ncls, :w], in_=pa[:c.ncls, :w])
                    for s in range(w // 128):
                        pb = pbpool.tile([128, c.ncls], F32)
                        nc.tensor.transpose(
                            out=pb[:, :c.ncls],
                            in_=stg[:c.ncls, s * 128:(s + 1) * 128],
                            identity=ident[:c.ncls, :c.ncls],
                        )
                        seg = blk * (c.vblk // 128) + s
                        nc.vector.tensor_copy(
                            out=t_sb[:, seg * 128:seg * 128 + c.ncls],
                            in_=pb[:, :c.ncls])

            # bias + gather indices ride the sync queue behind the et loads
            # (needed only once gathering starts)
            nc.sync.dma_start(out=b_t[:], in_=b_d[:])
            nc.sync.dma_start(out=gi_all[:], in_=gidx_d[:])

            # ---- phase 2+3: gather + reduce, chunked RS + softmax ----
            pooled = cpool.tile([128, c.batch], F32)
            maxw = c.gdocs * max(c.budgets)
            ngrp_chunk = c.ochunks and len(c.gorder) // c.ochunks
            with (
                tc.tile_pool(name="gw", bufs=8) as gwpool,
                tc.tile_pool(name="sm", bufs=2) as smpool,
                tc.tile_pool(name="sms", bufs=2) as sspool,
                tc.tile_pool(name="tps", bufs=2, space="PSUM") as tpool,
            ):
                def emit_partial(s, ch):
                    """One output-slice's partials for chunk ch -> DRAM, as
                    soon as its 4 rank-groups are pooled (keeps the RS
                    trigger's wait nearly satisfied at the chunk boundary)."""
                    col = s * c.docs_out + ch * 128
                    nc.sync.dma_start(
                        out=partials_d[ch][s * c.ncls:(s + 1) * c.ncls, :],
                        in_=pooled[:c.ncls, col:col + 128])

                def emit_chunk(ch):
                    """RS + bias + softmax for output docs [128ch, 128ch+128)
                    of every core's slice."""
                    nc.gpsimd.collective_compute(
                        "ReduceScatter",
                        mybir.AluOpType.add,
                        replica_groups=[list(range(NCORES))],
                        ins=[partials_d[ch][:]],
                        outs=[rs_d[ch][:]],
                    )
                    rs_sb = smpool.tile([c.ncls, 128], F32)
                    nc.sync.dma_start(out=rs_sb[:], in_=rs_d[ch][:])
                    tp = tpool.tile([128, c.ncls], F32)
                    nc.tensor.transpose(
                        out=tp[:, :c.ncls],
                        in_=rs_sb[:, :],
                        identity=ident[:c.ncls, :c.ncls],
                    )
                    lt = smpool.tile([128, c.ncls], F32)
                    nc.vector.tensor_tensor(out=lt[:], in0=tp[:], in1=b_t[:],
                                            op=mybir.AluOpType.add)
                    nmx = sspool.tile([128, 1], F32)
                    nc.vector.tensor_reduce(out=nmx[:], in_=lt[:],
                                            op=mybir.AluOpType.max,
                                            axis=mybir.AxisListType.X,
                                            negate=True)
                    ex = smpool.tile([128, c.ncls], F32)
                    nc.scalar.activation(out=ex[:], in_=lt[:],
                                         func=mybir.ActivationFunctionType.Exp,
                                         bias=nmx[:], scale=1.0)
                    sm = sspool.tile([128, 1], F32)
                    nc.vector.reduce_sum(out=sm[:], in_=ex[:],
                                         axis=mybir.AxisListType.X)
                    rc = sspool.tile([128, 1], F32)
                    nc.vector.reciprocal(out=rc[:], in_=sm[:])
                    ot = smpool.tile([128, c.ncls], F32)
                    nc.vector.tensor_scalar_mul(out=ot[:], in0=ex[:],
                                                scalar1=rc[:])
                    nc.sync.dma_start(out=out_d[ch * 128:(ch + 1) * 128, :],
                                      in_=ot[:])

                grp_done = -1
                groups_reduced = 0
                chunks_emitted = 0
                g_w = None
                g3 = None
                for (g, off, n, q) in c.calls:
                    if g != grp_done:
                        # new group: fresh tile, sliced to this group's width
                        b = c.budgets[g]
                        g_t = gwpool.tile([128, maxw], BF16)
                        g_w = g_t[:, :c.gdocs * b]
                        g3 = g_w.rearrange("p (s n) -> p s n", s=1)
                        grp_done = g
                    base = c.goff[g]
                    nc.gpsimd.dma_gather(
                        out_ap=g3[:, :, off:off + n],
                        in_ap=t_sb[:],
                        idxs_ap=gi_all[:, (base + off) // 16:
                                       (base + off + n) // 16],
                        num_idxs=n,
                        num_idxs_reg=n,
                        elem_size=128,
                        transpose=True,
                        single_packet=True,
                        queue_num=q,
                        sbuf_tokens_per_rank=128,
                        sbuf_free_dim_per_rank=256,
                        sbuf_free_dim_pad_per_rank=0,
                        sbuf_byte_offset=0,
                    )
                    if off + n == c.gdocs * c.budgets[g]:
                        g3d = g_w.rearrange("p (d t) -> p d t",
                                            t=c.budgets[g])
                        nc.vector.tensor_reduce(
                            out=pooled[:, g * c.gdocs:(g + 1) * c.gdocs],
                            in_=g3d,
                            op=mybir.AluOpType.add,
                            axis=mybir.AxisListType.X)
                        groups_reduced += 1
                        p = groups_reduced - 1
                        ch_p, rem = divmod(p, ngrp_chunk)
                        s_p, i_p = divmod(rem, c.gp_chunk)
                        if i_p == c.gp_chunk - 1:
                            emit_partial(s_p, ch_p)
                        # trigger chunk RS 8 groups into the NEXT chunk so
                        # its sem waits are satisfied and the gpsimd queue
                        # never head-of-line blocks on them
                        if groups_reduced == (chunks_emitted + 1) * ngrp_chunk + 8:
                            emit_chunk(chunks_emitted)
                            chunks_emitted += 1
                while chunks_emitted < c.ochunks:
                    emit_chunk(chunks_emitted)
                    chunks_emitted += 1
    nc.compile()
    return nc


def _plan(x: np.ndarray):
    """Doc ordering + per-group budgets from the actual token counts.

    Returns (cfg, order) where order[rank] = original doc id; docs are
    processed in rank order so each 32-doc group's budget is the exact
    (ceil-4) max token count over its docs and all cores."""
    x = np.asarray(x).astype(np.int64)
    B, L = x.shape
    vsh = 100000 // NCORES
    flat_v = x.reshape(-1)
    tok_doc = np.repeat(np.arange(B, dtype=np.int64), L)
    core_of = flat_v // vsh
    key = core_of * B + tok_doc
    counts = np.bincount(key, minlength=NCORES * B).reshape(NCORES, B)
    docmax = counts.max(axis=0)
    order = np.argsort(docmax, kind="stable")
    smax = docmax[order]
    budgets = [_ceil4(smax[g * 32 + 31]) for g in range(B // 32)]
    cfg = Cfg(budgets, batch=B, doclen=L)
    return cfg, order


def _prep_index_inputs(cfg: Cfg, x: np.ndarray, order: np.ndarray):
    """Rank-major gather indices (16-wrap int16 per call).
    Returns gidx[8, 128, nslots/16]."""
    c = cfg
    x = np.asarray(x).astype(np.int64)
    flat_v = x.reshape(-1)
    tok_doc = np.repeat(np.arange(c.batch, dtype=np.int64), c.doclen)
    core_of = flat_v // c.vsh
    local = (flat_v - core_of * c.vsh).astype(np.int64)

    rank_of = np.empty(c.batch, np.int64)
    rank_of[order] = np.arange(c.batch, dtype=np.int64)

    key = core_of * c.batch + tok_doc
    counts = np.bincount(key, minlength=NCORES * c.batch)
    ord_t = np.argsort(key, kind="stable")
    key_s = key[ord_t]
    group_start = np.zeros(NCORES * c.batch, np.int64)
    np.cumsum(counts[:-1], out=group_start[1:])
    pos = np.arange(key.size, dtype=np.int64) - group_start[key_s]
    core_s = key_s // c.batch
    rank_s = rank_of[key_s % c.batch]

    g = rank_s // c.gdocs
    budg = np.asarray(c.budgets, np.int64)
    goff = np.asarray(c.goff[:-1], np.int64)
    slot = goff[g] + (rank_s % c.gdocs) * budg[g] + pos
    assert (pos < budg[g]).all()

    # pads round-robin over the 128 zero rows of the spare segment
    gflat = np.broadcast_to(
        c.pad_idx + (np.arange(c.nslots, dtype=np.int64) % 128),
        (NCORES, c.nslots)).copy()
    gflat[core_s, slot] = local[ord_t]

    # 16-wrap per call: within each call, token j -> [j%16, j//16]
    g16 = np.empty((NCORES, 16, c.nslots // 16), np.int16)
    for (grp, off, n, _q) in c.calls:
        a = c.goff[grp] + off
        seg = gflat[:, a:a + n].reshape(NCORES, n // 16, 16)
        g16[:, :, a // 16:(a + n) // 16] = seg.transpose(0, 2, 1)
    gidx = np.tile(g16, (1, 8, 1)).astype(np.int16)   # (8, 128, cols)
    return gidx


_PROGRAM_CACHE: dict = {}


def _get_program(cfg: Cfg):
    k = cfg.key()
    if k not in _PROGRAM_CACHE:
        _PROGRAM_CACHE[k] = _build_program(cfg)
    return _PROGRAM_CACHE[k]


def run(embeddings, W, b, x, trace=False, tmpdir=None):
    embeddings = np.ascontiguousarray(np.asarray(embeddings, dtype=np.float32))
    W = np.ascontiguousarray(np.asarray(W, dtype=np.float32))
    b = np.asarray(b, dtype=np.float32).reshape(1, -1)
    x = np.asarray(x)

    cfg, order = _plan(x)
    gidx = _prep_index_inputs(cfg, x, order)
    nc = _get_program(cfg)

    wt_host = np.zeros((cfg.kpad, cfg.ncls), np.float32)
    wt_host[:cfg.embed] = W.T
    b_tiled = np.tile(b, (128, 1)).astype(np.float32)
    in_maps = []
    for c in range(NCORES):
        e_pad = np.zeros((cfg.vpad, cfg.embed), np.float32)
        e_pad[:cfg.vsh] = embeddings[c * cfg.vsh:(c + 1) * cfg.vsh]
        et = np.zeros((cfg.kpad, cfg.vpad), ml_dtypes.bfloat16)
        et[:cfg.embed] = e_pad.T.astype(ml_dtypes.bfloat16)
        in_maps.append({
            "et_d": et,
            "wt_d": wt_host,
            "b_d": b_tiled,
            "gidx_d": gidx[c],
        })
    res = run_bass_kernel_spmd(nc, in_maps, list(range(NCORES)),
                               trace=trace, tmpdir=tmpdir)
    ranked = np.concatenate([res.results[c]["out"] for c in range(NCORES)],
                            axis=0)
    out = np.empty_like(ranked)
    out[order] = ranked          # rank r held doc order[r]
    return out, res


def kernel(embeddings, W, b, x):
    out, _ = run(embeddings, W, b, x)
    return out
